# revision 1
# baseline (speedup 1.0000x reference)
"""Trainium2 Bass kernel for nn_BRNN_8151847927833.

Binary RNN: B=64 seqs, T=512 steps, d_model=1024, 6 binary FF layers per
step, then a small head + log_softmax + NLL loss averaged over (t, b).

Strategy (data-parallel over batch, 8 cores x 8 sequences):
  - All weights are +-1 (sign of latents), thresholds are small integers.
    Matmuls are therefore EXACT in low precision: products are +-1 and
    PSUM accumulates in fp32.
  - Activations are kept transposed: x^T stored as [128 partitions, 64]
    where column = m_chunk*8 + b (8 chunks of 128 dims x 8 batch).
    Weight-stationary matmuls (lhsT = W chunk [128k x 128m], moving
    rhs = x^T chunk [128, 8]) produce the NEXT transposed layout
    directly -> zero transposes in the whole recurrence.
  - Activations use a {0,1} encoding (h' = (h+1)/2) so the per-layer
    nonlinearity is a single DVE is_ge against a host-folded threshold
    (thr + colsum(W))/2 — exact integer-vs-half-integer compare, and no
    ScalarE hop on the recurrence critical path.  The activation is
    split per 128-dim chunk so each chunk unblocks the next layer's
    matmuls as soon as its PSUM accumulation group completes.
  - The head + log-softmax + token-gather do NOT feed the recurrence, so
    they are deferred: the 128 "read" dims per step are stored to a
    [128, T*8] buffer and processed as 32 dense batched matmul tiles
    after the T-loop.  No max-subtraction needed: |logits| <= 8.
  - Each core returns per-partition partial sums of (logsumexp - logit_tok);
    the host sums across cores and divides by B*T.
"""

import math
import sys

import numpy as np

sys.path.insert(0, "/opt/trn_rl_repo")

import ml_dtypes  # noqa: E402

import concourse.bass as bass  # noqa: E402
import concourse.bacc as bacc  # noqa: E402
import concourse.mybir as mybir  # noqa: E402
from concourse.tile import TileContext  # noqa: E402
from concourse.bass_utils import run_bass_kernel_spmd  # noqa: E402

F32 = mybir.dt.float32
BF16 = mybir.dt.bfloat16
FP8 = mybir.dt.float8e4
NP_BF16 = ml_dtypes.bfloat16
NP_FP8 = ml_dtypes.float8_e4m3

D = 1024          # d_model
KC = 8            # contraction chunks of 128
MC = 8            # output chunks of 128
NUMFF = 6
VOCAB = 128
READ = 128
CARRY = 896
BL = 8            # batch per core
NCORES = 8
LOGIT_SCALE = 1.0 / 16.0

# weight dtype for the FF stack / head (fp8 -> fast weight load)
W_DT = FP8
W_NP = NP_FP8


def build_nc(T):
    """Build the SPMD Bass kernel for T timesteps (BL sequences/core)."""
    ntile = T * BL // 128  # tail tiles over (t, b)
    assert T * BL % 128 == 0

    nc = bacc.Bacc("TRN2", target_bir_lowering=False)
    wff = nc.dram_tensor("wff", [128, NUMFF * KC * D], W_DT, kind="ExternalInput")
    emb = nc.dram_tensor("emb", [128, T * BL], BF16, kind="ExternalInput")
    x0 = nc.dram_tensor("x0", [128, MC * BL], BF16, kind="ExternalInput")
    thr = nc.dram_tensor("thr", [128, NUMFF * MC * BL], F32, kind="ExternalInput")
    headw = nc.dram_tensor("headw", [128, VOCAB], W_DT, kind="ExternalInput")
    oneh = nc.dram_tensor("oneh", [128, ntile * VOCAB], F32, kind="ExternalInput")
    wexp = nc.dram_tensor("wexp", [128, 4 * VOCAB], F32, kind="ExternalInput")
    res = nc.dram_tensor("res", [128, 1], F32, kind="ExternalOutput")

    AT = mybir.ActivationFunctionType
    ALU = mybir.AluOpType

    with TileContext(nc) as tc:
        with (
            tc.tile_pool(name="const", bufs=1) as cpool,
            tc.tile_pool(name="work", bufs=4) as wpool,
            tc.tile_pool(name="hpool", bufs=3) as hpool,
            tc.tile_pool(name="pst", bufs=1, space="PSUM") as pstpool,
            tc.tile_pool(name="ps2pool", bufs=2, space="PSUM") as ps2pool,
        ):
            # ---- resident inputs (DMA in consumption order) ----
            xb = cpool.tile([128, MC * BL], BF16, tag="xb")
            nc.sync.dma_start(out=xb[:, :], in_=x0[:, :])
            thrsb = cpool.tile([128, NUMFF * MC * BL], F32, tag="thrsb")
            nc.sync.dma_start(out=thrsb[:, :], in_=thr[:, :])
            # per-layer weight tiles so step 0 only waits for layer 0
            wsbs = []
            for i in range(NUMFF):
                w_i = cpool.tile([128, KC * D], W_DT, name=f"wsb{i}", tag=f"wsb{i}")
                nc.sync.dma_start(
                    out=w_i[:, :], in_=wff[:, i * KC * D : (i + 1) * KC * D]
                )
                wsbs.append(w_i)
            embsb = cpool.tile([128, T * BL], BF16, tag="embsb")
            nc.sync.dma_start(out=embsb[:, :], in_=emb[:, :])
            headsb = cpool.tile([128, VOCAB], W_DT, tag="headsb")
            nc.sync.dma_start(out=headsb[:, :], in_=headw[:, :])
            onehsb = cpool.tile([128, ntile * VOCAB], F32, tag="onehsb")
            nc.sync.dma_start(out=onehsb[:, :], in_=oneh[:, :])
            wexpsb = cpool.tile([128, 4 * VOCAB], F32, tag="wexpsb")
            nc.sync.dma_start(out=wexpsb[:, :], in_=wexp[:, :])

            # ---- persistent state ----
            readst = cpool.tile([128, T * BL], BF16, tag="readst")
            sumexp = cpool.tile([128, ntile], F32, tag="sumexp")
            tokl = cpool.tile([128, max(1, ntile // 4)], F32, tag="tokl")
            # 4 cycling PSUM tiles (one bank each) so DVE activation reads
            # never share a bank with in-flight PE writes (reuse distance
            # = 4 accumulation groups)
            NPS = 4
            psts = [
                pstpool.tile([128, MC * BL // NPS], F32, name=f"pst{j}", tag=f"pst{j}")
                for j in range(NPS)
            ]

            # ---- recurrence ----
            # Matmul emission order per layer: groups 0 and 1 emit k=0..6
            # first (14 pairs independent of the previous layer's final
            # activation chunk), then their k=7 finishers, then groups 2..7.
            # This covers the previous layer's last-chunk activation latency.
            mm_order = (
                [(0, k) for k in range(KC - 1)]
                + [(1, k) for k in range(KC - 1)]
                + [(0, KC - 1), (1, KC - 1)]
            )
            for m in range(2, MC):
                mm_order += [(m, k) for k in range(KC)]

            def emit_act(nc, i, m, src_ps, pc, dst):
                nc.vector.scalar_tensor_tensor(
                    out=dst,
                    in0=src_ps[:, pc : pc + BL],
                    scalar=0.0,
                    in1=thrsb[:, i * MC * BL + m * BL : i * MC * BL + (m + 1) * BL],
                    op0=ALU.add,
                    op1=ALU.is_ge,
                )

            for t in range(T):
                src = xb
                for i in range(NUMFF):
                    last = i == NUMFF - 1
                    wsb = wsbs[i]
                    h = None if last else hpool.tile([128, MC * BL], BF16, tag="h")

                    def dst_for(m):
                        if last:
                            if m == MC - 1:
                                return readst[:, t * BL : (t + 1) * BL]
                            return xb[:, m * BL : (m + 1) * BL]
                        return h[:, m * BL : (m + 1) * BL]

                    for m, k in mm_order:
                        ps = psts[m % NPS]
                        pc = (m // NPS) * BL
                        lo = m * 128
                        nc.tensor.matmul(
                            ps[:, pc : pc + BL],
                            wsb[:, k * D + lo : k * D + lo + 128],
                            src[:, k * BL : (k + 1) * BL],
                            start=(k == 0),
                            stop=(k == KC - 1),
                        )
                        if k == KC - 1:
                            # group m complete: per-chunk activation
                            # h'[m] = (pre' >= thr2) in {1,0}
                            emit_act(nc, i, m, ps, pc, dst_for(m))
                    if i == 0:
                        # refill the embed chunk of x for the NEXT step as
                        # soon as this step's layer-0 matmuls consumed it
                        nc.vector.tensor_copy(
                            xb[:, CARRY // 16 : MC * BL],
                            embsb[:, t * BL : (t + 1) * BL],
                        )
                    if not last:
                        src = h

            # ---- deferred head + loss (4 tiles batched per PSUM bank) ----
            assert ntile % 4 == 0 or ntile < 4
            nj = max(1, ntile // 4)
            per = min(4, ntile)
            for j in range(nj):
                ps2 = ps2pool.tile([128, per * VOCAB], F32, tag="ps2")
                for u in range(per):
                    nc.tensor.matmul(
                        ps2[:, u * VOCAB : (u + 1) * VOCAB],
                        readst[:, (j * per + u) * 128 : (j * per + u + 1) * 128],
                        headsb[:, :],
                        start=True,
                        stop=True,
                    )
                # logits = (2*ps2 - colsum(H)) / 16 ; exp(logits) =
                # exp(ps2/8) * wexp  with wexp = exp(-colsum(H)/16)
                expt = wpool.tile([128, per * VOCAB], F32, tag="expt")
                nc.scalar.activation(
                    expt[:, :],
                    ps2[:, :],
                    AT.Exp,
                    scale=2.0 * LOGIT_SCALE,
                )
                junk2 = wpool.tile([128, per * VOCAB], F32, tag="junk2")
                for u in range(per):
                    nc.vector.scalar_tensor_tensor(
                        out=junk2[:, u * VOCAB : (u + 1) * VOCAB],
                        in0=expt[:, u * VOCAB : (u + 1) * VOCAB],
                        scalar=1.0,
                        in1=wexpsb[:, 0:VOCAB],
                        op0=ALU.mult,
                        op1=ALU.mult,
                        accum_out=sumexp[:, j * per + u : j * per + u + 1],
                    )
                # device part of logit_tok: (2/16) * ps2_tok (host adds the
                # -colsum(H)[tok]/16 correction)
                junk = wpool.tile([128, per * VOCAB], F32, tag="junk")
                nc.vector.scalar_tensor_tensor(
                    out=junk[:, :],
                    in0=ps2[:, :],
                    scalar=2.0 * LOGIT_SCALE,
                    in1=onehsb[:, j * per * VOCAB : (j + 1) * per * VOCAB],
                    op0=ALU.mult,
                    op1=ALU.mult,
                    accum_out=tokl[:, j : j + 1],
                )

            lse = cpool.tile([128, ntile], F32, tag="lse")
            nc.scalar.activation(lse[:, :], sumexp[:, :], AT.Ln)
            r1 = cpool.tile([128, 1], F32, tag="r1")
            r2 = cpool.tile([128, 1], F32, tag="r2")
            po = cpool.tile([128, 1], F32, tag="po")
            nc.vector.tensor_reduce(
                r1[:, :], lse[:, :], axis=mybir.AxisListType.X, op=ALU.add
            )
            nc.vector.tensor_reduce(
                r2[:, :], tokl[:, :], axis=mybir.AxisListType.X, op=ALU.add
            )
            nc.vector.scalar_tensor_tensor(
                out=po[:, :],
                in0=r1[:, :],
                scalar=0.0,
                in1=r2[:, :],
                op0=ALU.add,
                op1=ALU.subtract,
            )
            nc.sync.dma_start(out=res[:, :], in_=po[:, :])

    return nc


def _sgn(a):
    return np.where(np.asarray(a, np.float32) >= 0, 1.0, -1.0).astype(np.float32)


def prepare_inputs(tokens, initial_lat, embed_lat, ff_lat, head_lat, ff_thresh_lat):
    """Host-side transform of the full problem inputs to per-core DRAM maps.

    Activations are sent in {0,1} encoding h' = (h+1)/2, with thresholds
    folded:  pre >= thr  <=>  h'@W >= (thr + colsum(W))/2.
    Returns (in_maps, B, T, host_corr) where host_corr is the token-logit
    correction  sum_{b,t} colsum(H)[tok]/16  to add to the loss sum.
    """
    tokens = np.asarray(tokens).astype(np.int64)
    B, T = tokens.shape
    E = _sgn(embed_lat)                      # [V, 128]
    W = _sgn(ff_lat)                         # [6, 1024, 1024]
    H = _sgn(head_lat)                       # [128, V]
    x0v = _sgn(initial_lat)                  # [1024]
    R = np.round(np.asarray(ff_thresh_lat, np.float64)).astype(np.float32)  # [6,1024]

    # weights: wff[p, (i*KC+k)*D + mcol] = W[i, k*128+p, mcol]
    wff = (
        W.reshape(NUMFF, KC, 128, D).transpose(2, 0, 1, 3).reshape(128, NUMFF * KC * D)
    ).astype(W_NP)
    headw = H.astype(W_NP)                   # [128 r, V]

    # x0 in {0,1}: col = chunk*8 + b, value (x0v+1)/2 replicated over b
    x0t = ((x0v + 1.0) / 2.0).reshape(MC, 128).T     # [p, chunk]
    x0arr = np.repeat(x0t[:, :, None], BL, axis=2).reshape(128, MC * BL).astype(NP_BF16)

    # folded threshold thr2 = (thr + colsum(W))/2, expanded col = i*64+m*8+b
    S = W.sum(axis=1)                        # [6, 1024] colsums
    thr2 = (R + S) / 2.0
    thr2 = thr2.reshape(NUMFF, MC, 128).transpose(2, 0, 1)[:, :, :, None]
    thr2 = np.broadcast_to(thr2, (128, NUMFF, MC, BL)).reshape(128, NUMFF * MC * BL)
    thr2 = np.ascontiguousarray(thr2, np.float32)

    # head colsum corrections
    csH = H.sum(axis=0)                      # [V]
    wexp = np.exp(-csH / 16.0).astype(np.float32)
    wexp = np.ascontiguousarray(np.tile(wexp[None, :], (128, 4)))
    host_corr = float(csH[tokens].sum()) / 16.0

    ntile = T * BL // 128
    in_maps = []
    for c in range(NCORES):
        tc_ = tokens[c * BL : (c + 1) * BL]  # [8, T]
        # emb in {0,1}: emb[p, t*8+b] = (E[tok[b,t], p]+1)/2
        embc = ((E[tc_] + 1.0) / 2.0).transpose(2, 1, 0).reshape(128, T * BL)
        embc = embc.astype(NP_BF16)
        # one-hot over vocab per (t,b) sample, tiled [128 samples x 128 vocab]
        flat = tc_.T.reshape(-1)             # tb = t*8+b
        onehc = (flat[:, None] == np.arange(VOCAB)[None, :]).astype(np.float32)
        onehc = (
            onehc.reshape(ntile, 128, VOCAB).transpose(1, 0, 2).reshape(128, ntile * VOCAB)
        )
        onehc = np.ascontiguousarray(onehc)
        in_maps.append(
            {
                "wff": wff,
                "emb": embc,
                "x0": x0arr,
                "thr": thr2,
                "headw": headw,
                "oneh": onehc,
                "wexp": wexp,
            }
        )
    return in_maps, B, T, host_corr


def _install_axon_trace_hook():
    """The image's antenv lacks axon_hooks; recreate the NTFF profile hook
    via ctypes against libaxon_pjrt.so (mirrors trn_agent_boot.trn_boot)."""
    import contextlib
    import ctypes
    import types

    try:
        from antenv.axon_hooks import get_axon_ntff_profile_hook  # noqa: F401

        return
    except ImportError:
        pass
    so_path = "/opt/axon/libaxon_pjrt.so"
    lib = ctypes.CDLL(so_path)
    if not hasattr(lib, "axon_start_nrt_profile"):
        return
    lib.axon_start_nrt_profile.argtypes = [
        ctypes.POINTER(ctypes.c_int64),
        ctypes.c_size_t,
    ]
    lib.axon_start_nrt_profile.restype = ctypes.c_int64
    lib.axon_stop_nrt_profile.argtypes = [ctypes.c_char_p]
    lib.axon_stop_nrt_profile.restype = ctypes.c_int64

    @contextlib.contextmanager
    def _hook(output_dir, device_ids):
        import jax

        jax.devices()
        if device_ids:
            ids = (ctypes.c_int64 * len(device_ids))(*device_ids)
            rc = lib.axon_start_nrt_profile(ids, len(device_ids))
        else:
            rc = lib.axon_start_nrt_profile(None, 0)
        if rc != 0:
            raise RuntimeError(f"axon_start_nrt_profile rc={rc}")
        try:
            yield
        finally:
            n = lib.axon_stop_nrt_profile(str(output_dir).encode())
            print(f"profile: {n} file(s) written to {output_dir}", file=sys.stderr)

    import antenv

    mod = types.ModuleType("antenv.axon_hooks")
    mod.get_axon_ntff_profile_hook = lambda: _hook
    sys.modules["antenv.axon_hooks"] = mod
    antenv.axon_hooks = mod

    from concourse import bass_utils as bu

    bu.upload_artifacts = lambda tmpdir: f"local://{tmpdir}"


def run(trace=False, tmpdir=None, **inputs):
    in_maps, B, T, host_corr = prepare_inputs(**inputs)
    nc = build_nc(T)
    if not nc.is_finalized():
        nc.finalize()
    if trace:
        _install_axon_trace_hook()
    out = run_bass_kernel_spmd(
        nc, in_maps, core_ids=list(range(NCORES)), trace=trace, tmpdir=tmpdir
    )
    total = host_corr
    for r in out.results:
        total += np.asarray(r["res"], np.float64).sum()
    loss = np.float32(total / (B * T))
    return np.asarray(loss, dtype=np.float32), out


def kernel(**inputs):
    loss, _ = run(trace=False, **inputs)
    return loss


if __name__ == "__main__":
    # tiny smoke test
    import jax

    sys.path.insert(0, "/root/problem")
    import reference

    inputs = reference.setup_inputs()
    inputs = {k: np.asarray(v) for k, v in inputs.items()}
    Tsmall = int(sys.argv[1]) if len(sys.argv) > 1 else 16
    inputs["tokens"] = inputs["tokens"][:, :Tsmall]
    expected = np.asarray(reference.reference(**{k: v for k, v in inputs.items()}))
    got = kernel(**inputs)
    rel = abs(float(got) - float(expected)) / max(1e-12, abs(float(expected)))
    print(f"T={Tsmall} expected={expected} got={got} rel_err={rel:.3e}")



# revision 2
# speedup vs baseline: 1.2190x; 1.2190x over previous
"""Trainium2 Bass kernel for nn_BRNN_8151847927833 — v2.

Binary RNN: B=64 seqs, T=512 steps, d_model=1024, 6 binary FF layers per
step, then a small head + log_softmax + NLL loss averaged over (t, b).

v2 changes vs baseline (which was LDWEIGHTS-feed-bound at ~32ns/pair with
~350ns/layer of PE stall on per-chunk act semaphores):
  - One [128, 4096] f32 PSUM tile spanning all 8 banks; accumulation
    group m lives in bank m (cols 512*m + 8*parity), so all 8 groups can
    be open concurrently and the matmul emission order is free.
  - Activations collapsed from 8 tiny DVE ops/layer to 3 wide ones
    (groups {0,1,2}, {3,4,5}, {6,7}) using 3D cross-bank strided APs.
    Fewer PE semaphore waits, less DVE pressure.
  - Emission order staggers group completion (group 0's k6/k7 finishers
    deferred ~12 pairs) so the next layer's stream never stalls on the
    final activation of the previous layer.
  - Layer-0 k=7 matmuls read the embed buffer directly (per-step DVE
    copy removed).
  - Head phase reuses the same PSUM banks after the T-loop.
"""

import math
import sys

import numpy as np

sys.path.insert(0, "/opt/trn_rl_repo")

import ml_dtypes  # noqa: E402

import concourse.bass as bass  # noqa: E402
import concourse.bacc as bacc  # noqa: E402
import concourse.mybir as mybir  # noqa: E402
from concourse.tile import TileContext  # noqa: E402
from concourse.bass_utils import run_bass_kernel_spmd  # noqa: E402

F32 = mybir.dt.float32
BF16 = mybir.dt.bfloat16
FP8 = mybir.dt.float8e4
NP_BF16 = ml_dtypes.bfloat16
NP_FP8 = ml_dtypes.float8_e4m3

D = 1024          # d_model
KC = 8            # contraction chunks of 128
MC = 8            # output chunks of 128
NUMFF = 6
VOCAB = 128
READ = 128
CARRY = 896
BL = 8            # batch per core
NCORES = 8
LOGIT_SCALE = 1.0 / 16.0
BANK = 512        # f32 cols per PSUM bank

W_DT = FP8
W_NP = NP_FP8


def build_nc(T):
    ntile = T * BL // 128  # tail tiles over (t, b)
    assert T * BL % 128 == 0

    nc = bacc.Bacc("TRN2", target_bir_lowering=False)
    wff = nc.dram_tensor("wff", [128, NUMFF * KC * D], W_DT, kind="ExternalInput")
    emb = nc.dram_tensor("emb", [128, T * BL], BF16, kind="ExternalInput")
    x0 = nc.dram_tensor("x0", [128, MC * BL], BF16, kind="ExternalInput")
    thr = nc.dram_tensor("thr", [128, NUMFF * MC * BL], F32, kind="ExternalInput")
    headw = nc.dram_tensor("headw", [128, VOCAB], W_DT, kind="ExternalInput")
    oneh = nc.dram_tensor("oneh", [128, ntile * VOCAB], F32, kind="ExternalInput")
    wexp = nc.dram_tensor("wexp", [128, 4 * VOCAB], F32, kind="ExternalInput")
    res = nc.dram_tensor("res", [128, 1], F32, kind="ExternalOutput")

    AT = mybir.ActivationFunctionType
    ALU = mybir.AluOpType

    with TileContext(nc) as tc:
        with (
            tc.tile_pool(name="const", bufs=1) as cpool,
            tc.tile_pool(name="work", bufs=4) as wpool,
            tc.tile_pool(name="hpool", bufs=3) as hpool,
            tc.tile_pool(name="pst", bufs=1, space="PSUM") as pstpool,
        ):
            # ---- resident inputs (DMA in consumption order) ----
            xb = cpool.tile([128, MC * BL], BF16, tag="xb")
            nc.sync.dma_start(out=xb[:, :], in_=x0[:, :])
            thrsb = cpool.tile([128, NUMFF * MC * BL], F32, tag="thrsb")
            nc.sync.dma_start(out=thrsb[:, :], in_=thr[:, :])
            wsbs = []
            for i in range(NUMFF):
                w_i = cpool.tile([128, KC * D], W_DT, name=f"wsb{i}", tag=f"wsb{i}")
                nc.sync.dma_start(
                    out=w_i[:, :], in_=wff[:, i * KC * D : (i + 1) * KC * D]
                )
                wsbs.append(w_i)
            embsb = cpool.tile([128, T * BL], BF16, tag="embsb")
            nc.sync.dma_start(out=embsb[:, :], in_=emb[:, :])
            headsb = cpool.tile([128, VOCAB], W_DT, tag="headsb")
            nc.sync.dma_start(out=headsb[:, :], in_=headw[:, :])
            onehsb = cpool.tile([128, ntile * VOCAB], F32, tag="onehsb")
            nc.sync.dma_start(out=onehsb[:, :], in_=oneh[:, :])
            wexpsb = cpool.tile([128, 4 * VOCAB], F32, tag="wexpsb")
            nc.sync.dma_start(out=wexpsb[:, :], in_=wexp[:, :])

            # ---- persistent state ----
            readst = cpool.tile([128, T * BL], BF16, tag="readst")
            sumexp = cpool.tile([128, ntile], F32, tag="sumexp")
            tokl = cpool.tile([128, max(1, ntile // 4)], F32, tag="tokl")
            # all 8 PSUM banks as one tile: group m -> bank m, layer-parity
            # picks cols 8p..8p+8 inside the bank so consecutive layers never
            # share a zero-region column range.
            psall = pstpool.tile([128, 8 * BANK], F32, tag="psall")
            psv = psall.rearrange("p (b c) -> p b c", b=8)  # [128, bank, 512]

            # per-layer matmul emission order: stagger group completion and
            # defer group 0/1 k>=6 finishers so the next layer's first pairs
            # never wait on act-c of the previous layer (which lands ~300ns
            # after the layer's last matmul).
            mm_order = (
                [(0, k) for k in range(6)]
                + [(1, k) for k in range(4)]
                + [(0, 6), (0, 7)]
                + [(1, k) for k in range(4, 8)]
                + [(2, k) for k in range(8)]
                + [(m, k) for m in range(3, 8) for k in range(8)]
            )
            assert len(mm_order) == 64

            def emit_act(i, p, g0, g1, dst):
                # dst <- (psum[groups g0:g1] >= thr) in {1,0}
                nc.vector.scalar_tensor_tensor(
                    out=dst,
                    in0=psv[:, g0:g1, 8 * p : 8 * p + 8],
                    scalar=0.0,
                    in1=thrsb[:, i * MC * BL + g0 * BL : i * MC * BL + g1 * BL],
                    op0=ALU.add,
                    op1=ALU.is_ge,
                )

            for t in range(T):
                src = xb          # layer-0 input (chunks 0..6; chunk 7 = emb)
                for i in range(NUMFF):
                    last = i == NUMFF - 1
                    wsb = wsbs[i]
                    p = (t * NUMFF + i) % 2  # layer parity for PSUM cols
                    h = None if last else hpool.tile([128, MC * BL], BF16, tag="h")

                    for m, k in mm_order:
                        if i == 0 and k == 7:
                            if t == 0:
                                rhs = xb[:, 7 * BL : 8 * BL]
                            else:
                                rhs = embsb[:, (t - 1) * BL : t * BL]
                        else:
                            rhs = src[:, k * BL : (k + 1) * BL]
                        nc.tensor.matmul(
                            psall[:, m * BANK + 8 * p : m * BANK + 8 * p + 8],
                            wsb[:, k * D + m * 128 : k * D + (m + 1) * 128],
                            rhs,
                            start=(k == 0),
                            stop=(k == KC - 1),
                        )
                    if last:
                        # carry chunks 0..6 -> xb, read chunk 7 -> readst
                        emit_act(i, p, 0, 3, xb[:, 0:24])
                        emit_act(i, p, 3, 6, xb[:, 24:48])
                        emit_act(i, p, 6, 7, xb[:, 48:56])
                        emit_act(i, p, 7, 8, readst[:, t * BL : (t + 1) * BL])
                    else:
                        emit_act(i, p, 0, 3, h[:, 0:24])
                        emit_act(i, p, 3, 6, h[:, 24:48])
                        emit_act(i, p, 6, 8, h[:, 48:64])
                        src = h

            # ---- deferred head + loss (4 tiles batched per PSUM bank) ----
            assert ntile % 4 == 0 or ntile < 4
            nj = max(1, ntile // 4)
            per = min(4, ntile)
            for j in range(nj):
                ps2 = psall[:, (j % 2) * BANK : (j % 2) * BANK + per * VOCAB]
                for u in range(per):
                    nc.tensor.matmul(
                        ps2[:, u * VOCAB : (u + 1) * VOCAB],
                        readst[:, (j * per + u) * 128 : (j * per + u + 1) * 128],
                        headsb[:, :],
                        start=True,
                        stop=True,
                    )
                expt = wpool.tile([128, per * VOCAB], F32, tag="expt")
                nc.scalar.activation(
                    expt[:, :],
                    ps2[:, :],
                    AT.Exp,
                    scale=2.0 * LOGIT_SCALE,
                )
                junk2 = wpool.tile([128, per * VOCAB], F32, tag="junk2")
                for u in range(per):
                    nc.vector.scalar_tensor_tensor(
                        out=junk2[:, u * VOCAB : (u + 1) * VOCAB],
                        in0=expt[:, u * VOCAB : (u + 1) * VOCAB],
                        scalar=1.0,
                        in1=wexpsb[:, 0:VOCAB],
                        op0=ALU.mult,
                        op1=ALU.mult,
                        accum_out=sumexp[:, j * per + u : j * per + u + 1],
                    )
                junk = wpool.tile([128, per * VOCAB], F32, tag="junk")
                nc.vector.scalar_tensor_tensor(
                    out=junk[:, :],
                    in0=ps2[:, :],
                    scalar=2.0 * LOGIT_SCALE,
                    in1=onehsb[:, j * per * VOCAB : (j + 1) * per * VOCAB],
                    op0=ALU.mult,
                    op1=ALU.mult,
                    accum_out=tokl[:, j : j + 1],
                )

            lse = cpool.tile([128, ntile], F32, tag="lse")
            nc.scalar.activation(lse[:, :], sumexp[:, :], AT.Ln)
            r1 = cpool.tile([128, 1], F32, tag="r1")
            r2 = cpool.tile([128, 1], F32, tag="r2")
            po = cpool.tile([128, 1], F32, tag="po")
            nc.vector.tensor_reduce(
                r1[:, :], lse[:, :], axis=mybir.AxisListType.X, op=ALU.add
            )
            nc.vector.tensor_reduce(
                r2[:, :], tokl[:, :], axis=mybir.AxisListType.X, op=ALU.add
            )
            nc.vector.scalar_tensor_tensor(
                out=po[:, :],
                in0=r1[:, :],
                scalar=0.0,
                in1=r2[:, :],
                op0=ALU.add,
                op1=ALU.subtract,
            )
            nc.sync.dma_start(out=res[:, :], in_=po[:, :])

    return nc


def _sgn(a):
    return np.where(np.asarray(a, np.float32) >= 0, 1.0, -1.0).astype(np.float32)


def prepare_inputs(tokens, initial_lat, embed_lat, ff_lat, head_lat, ff_thresh_lat):
    """Host-side transform of the full problem inputs to per-core DRAM maps.

    Activations are sent in {0,1} encoding h' = (h+1)/2, with thresholds
    folded:  pre >= thr  <=>  h'@W >= (thr + colsum(W))/2.
    """
    tokens = np.asarray(tokens).astype(np.int64)
    B, T = tokens.shape
    E = _sgn(embed_lat)                      # [V, 128]
    W = _sgn(ff_lat)                         # [6, 1024, 1024]
    H = _sgn(head_lat)                       # [128, V]
    x0v = _sgn(initial_lat)                  # [1024]
    R = np.round(np.asarray(ff_thresh_lat, np.float64)).astype(np.float32)  # [6,1024]

    wff = (
        W.reshape(NUMFF, KC, 128, D).transpose(2, 0, 1, 3).reshape(128, NUMFF * KC * D)
    ).astype(W_NP)
    headw = H.astype(W_NP)                   # [128 r, V]

    x0t = ((x0v + 1.0) / 2.0).reshape(MC, 128).T     # [p, chunk]
    x0arr = np.repeat(x0t[:, :, None], BL, axis=2).reshape(128, MC * BL).astype(NP_BF16)

    S = W.sum(axis=1)                        # [6, 1024] colsums
    thr2 = (R + S) / 2.0
    thr2 = thr2.reshape(NUMFF, MC, 128).transpose(2, 0, 1)[:, :, :, None]
    thr2 = np.broadcast_to(thr2, (128, NUMFF, MC, BL)).reshape(128, NUMFF * MC * BL)
    thr2 = np.ascontiguousarray(thr2, np.float32)

    csH = H.sum(axis=0)                      # [V]
    wexp = np.exp(-csH / 16.0).astype(np.float32)
    wexp = np.ascontiguousarray(np.tile(wexp[None, :], (128, 4)))
    host_corr = float(csH[tokens].sum()) / 16.0

    ntile = T * BL // 128
    in_maps = []
    for c in range(NCORES):
        tc_ = tokens[c * BL : (c + 1) * BL]  # [8, T]
        embc = ((E[tc_] + 1.0) / 2.0).transpose(2, 1, 0).reshape(128, T * BL)
        embc = embc.astype(NP_BF16)
        flat = tc_.T.reshape(-1)             # tb = t*8+b
        onehc = (flat[:, None] == np.arange(VOCAB)[None, :]).astype(np.float32)
        onehc = (
            onehc.reshape(ntile, 128, VOCAB).transpose(1, 0, 2).reshape(128, ntile * VOCAB)
        )
        onehc = np.ascontiguousarray(onehc)
        in_maps.append(
            {
                "wff": wff,
                "emb": embc,
                "x0": x0arr,
                "thr": thr2,
                "headw": headw,
                "oneh": onehc,
                "wexp": wexp,
            }
        )
    return in_maps, B, T, host_corr


def _install_axon_trace_hook():
    import contextlib
    import ctypes
    import types

    try:
        from antenv.axon_hooks import get_axon_ntff_profile_hook  # noqa: F401

        return
    except ImportError:
        pass
    so_path = "/opt/axon/libaxon_pjrt.so"
    lib = ctypes.CDLL(so_path)
    if not hasattr(lib, "axon_start_nrt_profile"):
        return
    lib.axon_start_nrt_profile.argtypes = [
        ctypes.POINTER(ctypes.c_int64),
        ctypes.c_size_t,
    ]
    lib.axon_start_nrt_profile.restype = ctypes.c_int64
    lib.axon_stop_nrt_profile.argtypes = [ctypes.c_char_p]
    lib.axon_stop_nrt_profile.restype = ctypes.c_int64

    @contextlib.contextmanager
    def _hook(output_dir, device_ids):
        import jax

        jax.devices()
        if device_ids:
            ids = (ctypes.c_int64 * len(device_ids))(*device_ids)
            rc = lib.axon_start_nrt_profile(ids, len(device_ids))
        else:
            rc = lib.axon_start_nrt_profile(None, 0)
        if rc != 0:
            raise RuntimeError(f"axon_start_nrt_profile rc={rc}")
        try:
            yield
        finally:
            n = lib.axon_stop_nrt_profile(str(output_dir).encode())
            print(f"profile: {n} file(s) written to {output_dir}", file=sys.stderr)

    import antenv

    mod = types.ModuleType("antenv.axon_hooks")
    mod.get_axon_ntff_profile_hook = lambda: _hook
    sys.modules["antenv.axon_hooks"] = mod
    antenv.axon_hooks = mod

    from concourse import bass_utils as bu

    bu.upload_artifacts = lambda tmpdir: f"local://{tmpdir}"


def run(trace=False, tmpdir=None, **inputs):
    in_maps, B, T, host_corr = prepare_inputs(**inputs)
    nc = build_nc(T)
    if not nc.is_finalized():
        nc.finalize()
    if trace:
        _install_axon_trace_hook()
    out = run_bass_kernel_spmd(
        nc, in_maps, core_ids=list(range(NCORES)), trace=trace, tmpdir=tmpdir
    )
    total = host_corr
    for r in out.results:
        total += np.asarray(r["res"], np.float64).sum()
    loss = np.float32(total / (B * T))
    return np.asarray(loss, dtype=np.float32), out


def kernel(**inputs):
    loss, _ = run(trace=False, **inputs)
    return loss


if __name__ == "__main__":
    import jax

    sys.path.insert(0, "/root/problem")
    import reference

    inputs = reference.setup_inputs()
    inputs = {k: np.asarray(v) for k, v in inputs.items()}
    Tsmall = int(sys.argv[1]) if len(sys.argv) > 1 else 16
    inputs["tokens"] = inputs["tokens"][:, :Tsmall]
    expected = np.asarray(reference.reference(**{k: v for k, v in inputs.items()}))
    got = kernel(**inputs)
    rel = abs(float(got) - float(expected)) / max(1e-12, abs(float(expected)))
    print(f"T={Tsmall} expected={expected} got={got} rel_err={rel:.3e}")


# revision 4
# speedup vs baseline: 1.2190x; 1.0001x over previous
"""Trainium2 Bass kernel for nn_BRNN_8151847927833 — v2.

Binary RNN: B=64 seqs, T=512 steps, d_model=1024, 6 binary FF layers per
step, then a small head + log_softmax + NLL loss averaged over (t, b).

v2 changes vs baseline (which was LDWEIGHTS-feed-bound at ~32ns/pair with
~350ns/layer of PE stall on per-chunk act semaphores):
  - One [128, 4096] f32 PSUM tile spanning all 8 banks; accumulation
    group m lives in bank m (cols 512*m + 8*parity), so all 8 groups can
    be open concurrently and the matmul emission order is free.
  - Activations collapsed from 8 tiny DVE ops/layer to 3 wide ones
    (groups {0,1,2}, {3,4,5}, {6,7}) using 3D cross-bank strided APs.
    Fewer PE semaphore waits, less DVE pressure.
  - Emission order staggers group completion (group 0's k6/k7 finishers
    deferred ~12 pairs) so the next layer's stream never stalls on the
    final activation of the previous layer.
  - Layer-0 k=7 matmuls read the embed buffer directly (per-step DVE
    copy removed).
  - Head phase reuses the same PSUM banks after the T-loop.
"""

import math
import sys

import numpy as np

sys.path.insert(0, "/opt/trn_rl_repo")

import ml_dtypes  # noqa: E402

import concourse.bass as bass  # noqa: E402
import concourse.bacc as bacc  # noqa: E402
import concourse.mybir as mybir  # noqa: E402
from concourse.tile import TileContext  # noqa: E402
from concourse.bass_utils import run_bass_kernel_spmd  # noqa: E402

F32 = mybir.dt.float32
BF16 = mybir.dt.bfloat16
FP8 = mybir.dt.float8e4
NP_BF16 = ml_dtypes.bfloat16
NP_FP8 = ml_dtypes.float8_e4m3

D = 1024          # d_model
KC = 8            # contraction chunks of 128
MC = 8            # output chunks of 128
NUMFF = 6
VOCAB = 128
READ = 128
CARRY = 896
BL = 8            # batch per core
NCORES = 8
LOGIT_SCALE = 1.0 / 16.0
BANK = 512        # f32 cols per PSUM bank

W_DT = FP8
W_NP = NP_FP8


def build_nc(T):
    ntile = T * BL // 128  # tail tiles over (t, b)
    assert T * BL % 128 == 0

    nc = bacc.Bacc("TRN2", target_bir_lowering=False)
    wff = nc.dram_tensor("wff", [128, NUMFF * KC * D], W_DT, kind="ExternalInput")
    emb = nc.dram_tensor("emb", [128, T * BL], BF16, kind="ExternalInput")
    x0 = nc.dram_tensor("x0", [128, MC * BL], BF16, kind="ExternalInput")
    thr = nc.dram_tensor("thr", [128, NUMFF * MC * BL], F32, kind="ExternalInput")
    headw = nc.dram_tensor("headw", [128, VOCAB], W_DT, kind="ExternalInput")
    oneh = nc.dram_tensor("oneh", [128, ntile * VOCAB], F32, kind="ExternalInput")
    wexp = nc.dram_tensor("wexp", [128, 4 * VOCAB], F32, kind="ExternalInput")
    res = nc.dram_tensor("res", [128, 1], F32, kind="ExternalOutput")

    AT = mybir.ActivationFunctionType
    ALU = mybir.AluOpType

    with TileContext(nc) as tc:
        with (
            tc.tile_pool(name="const", bufs=1) as cpool,
            tc.tile_pool(name="work", bufs=4) as wpool,
            tc.tile_pool(name="hpool", bufs=3) as hpool,
            tc.tile_pool(name="pst", bufs=1, space="PSUM") as pstpool,
        ):
            # ---- resident inputs (DMA in consumption order) ----
            xb = cpool.tile([128, MC * BL], BF16, tag="xb")
            nc.sync.dma_start(out=xb[:, :], in_=x0[:, :])
            thrsb = cpool.tile([128, NUMFF * MC * BL], F32, tag="thrsb")
            nc.sync.dma_start(out=thrsb[:, :], in_=thr[:, :])
            wsbs = []
            for i in range(NUMFF):
                w_i = cpool.tile([128, KC * D], W_DT, name=f"wsb{i}", tag=f"wsb{i}")
                nc.sync.dma_start(
                    out=w_i[:, :], in_=wff[:, i * KC * D : (i + 1) * KC * D]
                )
                wsbs.append(w_i)
            embsb = cpool.tile([128, T * BL], BF16, tag="embsb")
            nc.sync.dma_start(out=embsb[:, :], in_=emb[:, :])
            headsb = cpool.tile([128, VOCAB], W_DT, tag="headsb")
            nc.sync.dma_start(out=headsb[:, :], in_=headw[:, :])
            onehsb = cpool.tile([128, ntile * VOCAB], F32, tag="onehsb")
            nc.sync.dma_start(out=onehsb[:, :], in_=oneh[:, :])
            wexpsb = cpool.tile([128, 4 * VOCAB], F32, tag="wexpsb")
            nc.sync.dma_start(out=wexpsb[:, :], in_=wexp[:, :])

            # ---- persistent state ----
            readst = cpool.tile([128, T * BL], BF16, tag="readst")
            sumexp = cpool.tile([128, ntile], F32, tag="sumexp")
            tokl = cpool.tile([128, max(1, ntile // 4)], F32, tag="tokl")
            # all 8 PSUM banks as one tile: group m -> bank m, layer-parity
            # picks cols 8p..8p+8 inside the bank so consecutive layers never
            # share a zero-region column range.
            psall = pstpool.tile([128, 8 * BANK], F32, tag="psall")
            psv = psall.rearrange("p (b c) -> p b c", b=8)  # [128, bank, 512]

            # per-layer matmul emission order: stagger group completion and
            # defer group 0/1 k>=6 finishers so the next layer's first pairs
            # never wait on act-c of the previous layer (which lands ~300ns
            # after the layer's last matmul).
            mm_order = (
                [(0, k) for k in range(6)]
                + [(1, k) for k in range(4)]
                + [(0, 6), (0, 7)]
                + [(1, k) for k in range(4, 8)]
                + [(2, k) for k in range(8)]
                + [(m, k) for m in range(3, 8) for k in range(8)]
            )
            assert len(mm_order) == 64

            def emit_act(i, p, g0, g1, dst):
                # dst <- (psum[groups g0:g1] >= thr) in {1,0}
                nc.vector.scalar_tensor_tensor(
                    out=dst,
                    in0=psv[:, g0:g1, 8 * p : 8 * p + 8],
                    scalar=0.0,
                    in1=thrsb[:, i * MC * BL + g0 * BL : i * MC * BL + g1 * BL],
                    op0=ALU.add,
                    op1=ALU.is_ge,
                )

            for t in range(T):
                src = xb          # layer-0 input (chunks 0..6; chunk 7 = emb)
                for i in range(NUMFF):
                    last = i == NUMFF - 1
                    wsb = wsbs[i]
                    p = (t * NUMFF + i) % 2  # layer parity for PSUM cols
                    h = None if last else hpool.tile([128, MC * BL], BF16, tag="h")

                    for m, k in mm_order:
                        if i == 0 and k == 7:
                            if t == 0:
                                rhs = xb[:, 7 * BL : 8 * BL]
                            else:
                                rhs = embsb[:, (t - 1) * BL : t * BL]
                        else:
                            rhs = src[:, k * BL : (k + 1) * BL]
                        nc.tensor.matmul(
                            psall[:, m * BANK + 8 * p : m * BANK + 8 * p + 8],
                            wsb[:, k * D + m * 128 : k * D + (m + 1) * 128],
                            rhs,
                            start=(k == 0),
                            stop=(k == KC - 1),
                        )
                    if last:
                        # carry chunks 0..6 -> xb, read chunk 7 -> readst
                        emit_act(i, p, 0, 3, xb[:, 0:24])
                        emit_act(i, p, 3, 6, xb[:, 24:48])
                        emit_act(i, p, 6, 7, xb[:, 48:56])
                        emit_act(i, p, 7, 8, readst[:, t * BL : (t + 1) * BL])
                    else:
                        emit_act(i, p, 0, 3, h[:, 0:24])
                        emit_act(i, p, 3, 6, h[:, 24:48])
                        emit_act(i, p, 6, 8, h[:, 48:64])
                        src = h

            # ---- deferred head + loss (4 tiles batched per PSUM bank) ----
            assert ntile % 4 == 0 or ntile < 4
            nj = max(1, ntile // 4)
            per = min(4, ntile)
            for j in range(nj):
                ps2 = psall[:, (j % 2) * BANK : (j % 2) * BANK + per * VOCAB]
                for u in range(per):
                    nc.tensor.matmul(
                        ps2[:, u * VOCAB : (u + 1) * VOCAB],
                        readst[:, (j * per + u) * 128 : (j * per + u + 1) * 128],
                        headsb[:, :],
                        start=True,
                        stop=True,
                    )
                expt = wpool.tile([128, per * VOCAB], F32, tag="expt")
                nc.scalar.activation(
                    expt[:, :],
                    ps2[:, :],
                    AT.Exp,
                    scale=2.0 * LOGIT_SCALE,
                )
                junk2 = wpool.tile([128, per * VOCAB], F32, tag="junk2")
                for u in range(per):
                    nc.vector.scalar_tensor_tensor(
                        out=junk2[:, u * VOCAB : (u + 1) * VOCAB],
                        in0=expt[:, u * VOCAB : (u + 1) * VOCAB],
                        scalar=1.0,
                        in1=wexpsb[:, 0:VOCAB],
                        op0=ALU.mult,
                        op1=ALU.mult,
                        accum_out=sumexp[:, j * per + u : j * per + u + 1],
                    )
                junk = wpool.tile([128, per * VOCAB], F32, tag="junk")
                nc.vector.scalar_tensor_tensor(
                    out=junk[:, :],
                    in0=ps2[:, :],
                    scalar=2.0 * LOGIT_SCALE,
                    in1=onehsb[:, j * per * VOCAB : (j + 1) * per * VOCAB],
                    op0=ALU.mult,
                    op1=ALU.mult,
                    accum_out=tokl[:, j : j + 1],
                )

            lse = cpool.tile([128, ntile], F32, tag="lse")
            nc.scalar.activation(lse[:, :], sumexp[:, :], AT.Ln)
            r1 = cpool.tile([128, 1], F32, tag="r1")
            r2 = cpool.tile([128, 1], F32, tag="r2")
            po = cpool.tile([128, 1], F32, tag="po")
            nc.vector.tensor_reduce(
                r1[:, :], lse[:, :], axis=mybir.AxisListType.X, op=ALU.add
            )
            nc.vector.tensor_reduce(
                r2[:, :], tokl[:, :], axis=mybir.AxisListType.X, op=ALU.add
            )
            nc.vector.scalar_tensor_tensor(
                out=po[:, :],
                in0=r1[:, :],
                scalar=0.0,
                in1=r2[:, :],
                op0=ALU.add,
                op1=ALU.subtract,
            )
            nc.sync.dma_start(out=res[:, :], in_=po[:, :])

    return nc


def _sgn(a):
    return np.where(np.asarray(a, np.float32) >= 0, 1.0, -1.0).astype(np.float32)


def prepare_inputs(tokens, initial_lat, embed_lat, ff_lat, head_lat, ff_thresh_lat):
    """Host-side transform of the full problem inputs to per-core DRAM maps.

    Activations are sent in {0,1} encoding h' = (h+1)/2, with thresholds
    folded:  pre >= thr  <=>  h'@W >= (thr + colsum(W))/2.
    """
    tokens = np.asarray(tokens).astype(np.int64)
    B, T = tokens.shape
    E = _sgn(embed_lat)                      # [V, 128]
    W = _sgn(ff_lat)                         # [6, 1024, 1024]
    H = _sgn(head_lat)                       # [128, V]
    x0v = _sgn(initial_lat)                  # [1024]
    R = np.round(np.asarray(ff_thresh_lat, np.float64)).astype(np.float32)  # [6,1024]

    wff = (
        W.reshape(NUMFF, KC, 128, D).transpose(2, 0, 1, 3).reshape(128, NUMFF * KC * D)
    ).astype(W_NP)
    headw = H.astype(W_NP)                   # [128 r, V]

    x0t = ((x0v + 1.0) / 2.0).reshape(MC, 128).T     # [p, chunk]
    x0arr = np.repeat(x0t[:, :, None], BL, axis=2).reshape(128, MC * BL).astype(NP_BF16)

    S = W.sum(axis=1)                        # [6, 1024] colsums
    thr2 = (R + S) / 2.0
    thr2 = thr2.reshape(NUMFF, MC, 128).transpose(2, 0, 1)[:, :, :, None]
    thr2 = np.broadcast_to(thr2, (128, NUMFF, MC, BL)).reshape(128, NUMFF * MC * BL)
    thr2 = np.ascontiguousarray(thr2, np.float32)

    csH = H.sum(axis=0)                      # [V]
    wexp = np.exp(-csH / 16.0).astype(np.float32)
    wexp = np.ascontiguousarray(np.tile(wexp[None, :], (128, 4)))
    host_corr = float(csH[tokens].sum()) / 16.0

    ntile = T * BL // 128
    in_maps = []
    for c in range(NCORES):
        tc_ = tokens[c * BL : (c + 1) * BL]  # [8, T]
        embc = ((E[tc_] + 1.0) / 2.0).transpose(2, 1, 0).reshape(128, T * BL)
        embc = embc.astype(NP_BF16)
        flat = tc_.T.reshape(-1)             # tb = t*8+b
        onehc = (flat[:, None] == np.arange(VOCAB)[None, :]).astype(np.float32)
        onehc = (
            onehc.reshape(ntile, 128, VOCAB).transpose(1, 0, 2).reshape(128, ntile * VOCAB)
        )
        onehc = np.ascontiguousarray(onehc)
        in_maps.append(
            {
                "wff": wff,
                "emb": embc,
                "x0": x0arr,
                "thr": thr2,
                "headw": headw,
                "oneh": onehc,
                "wexp": wexp,
            }
        )
    return in_maps, B, T, host_corr


def _install_axon_trace_hook():
    import contextlib
    import ctypes
    import types

    try:
        from antenv.axon_hooks import get_axon_ntff_profile_hook  # noqa: F401

        return
    except ImportError:
        pass
    so_path = "/opt/axon/libaxon_pjrt.so"
    lib = ctypes.CDLL(so_path)
    if not hasattr(lib, "axon_start_nrt_profile"):
        return
    lib.axon_start_nrt_profile.argtypes = [
        ctypes.POINTER(ctypes.c_int64),
        ctypes.c_size_t,
    ]
    lib.axon_start_nrt_profile.restype = ctypes.c_int64
    lib.axon_stop_nrt_profile.argtypes = [ctypes.c_char_p]
    lib.axon_stop_nrt_profile.restype = ctypes.c_int64

    @contextlib.contextmanager
    def _hook(output_dir, device_ids):
        import jax

        jax.devices()
        if device_ids:
            ids = (ctypes.c_int64 * len(device_ids))(*device_ids)
            rc = lib.axon_start_nrt_profile(ids, len(device_ids))
        else:
            rc = lib.axon_start_nrt_profile(None, 0)
        if rc != 0:
            raise RuntimeError(f"axon_start_nrt_profile rc={rc}")
        try:
            yield
        finally:
            n = lib.axon_stop_nrt_profile(str(output_dir).encode())
            print(f"profile: {n} file(s) written to {output_dir}", file=sys.stderr)

    import antenv

    mod = types.ModuleType("antenv.axon_hooks")
    mod.get_axon_ntff_profile_hook = lambda: _hook
    sys.modules["antenv.axon_hooks"] = mod
    antenv.axon_hooks = mod

    from concourse import bass_utils as bu

    bu.upload_artifacts = lambda tmpdir: f"local://{tmpdir}"


def run(trace=False, tmpdir=None, **inputs):
    in_maps, B, T, host_corr = prepare_inputs(**inputs)
    nc = build_nc(T)
    if not nc.is_finalized():
        nc.finalize()
    if trace:
        _install_axon_trace_hook()
    out = run_bass_kernel_spmd(
        nc, in_maps, core_ids=list(range(NCORES)), trace=trace, tmpdir=tmpdir
    )
    total = host_corr
    for r in out.results:
        total += np.asarray(r["res"], np.float64).sum()
    loss = np.float32(total / (B * T))
    return np.asarray(loss, dtype=np.float32), out


def kernel(**inputs):
    loss, _ = run(trace=False, **inputs)
    return loss


if __name__ == "__main__":
    import jax

    sys.path.insert(0, "/root/problem")
    import reference

    inputs = reference.setup_inputs()
    inputs = {k: np.asarray(v) for k, v in inputs.items()}
    Tsmall = int(sys.argv[1]) if len(sys.argv) > 1 else 16
    inputs["tokens"] = inputs["tokens"][:, :Tsmall]
    expected = np.asarray(reference.reference(**{k: v for k, v in inputs.items()}))
    got = kernel(**inputs)
    rel = abs(float(got) - float(expected)) / max(1e-12, abs(float(expected)))
    print(f"T={Tsmall} expected={expected} got={got} rel_err={rel:.3e}")
